# revision 1
# baseline (speedup 1.0000x reference)
"""Trainium2 Bass kernel for nn_Jitter: per-timestep neighbor-replacement gather.

out[b, c, t] = x[b, c, g[t]] where
  g[t] = t                     if not replace_mask[t]
       = clamp-neighbor(t +/- 1) if replace_mask[t]   (t=0 -> 1, t=T-1 -> T-2)

g depends only on the tiny [T] vectors, so we precompute on the host two
per-timestep masks:
  pmask[t] = (g[t] == t-1)   -> take left neighbor
  nmask[t] = (g[t] == t+1)   -> take right neighbor
and on-device do, per [128, T] tile:
  out = copy(x)                           (Scalar/ACT engine)
  out[:,1:]  = where(pmask[1:],  x[:,:-1], out[:,1:])    (DVE copy_predicated)
  out[:,:-1] = where(nmask[:-1], x[:,1:],  out[:,:-1])   (DVE copy_predicated)

Sharding: pure data-parallel on batch; 8 cores x 4 batches each.
Each core's shard is [4*512, 4000] f32 = 16 tiles of [128, 4000].
"""

import numpy as np

import concourse.bass as bass
import concourse.tile as tile
from concourse import bacc, mybir
from concourse.bass_utils import run_bass_kernel_spmd

B, C, T = 32, 512, 4000
N_CORES = 8
B_PER = B // N_CORES            # 4 batches per core
ROWS = B_PER * C                # 2048 rows per core
P = 128                         # SBUF partitions
N_TILES = ROWS // P             # 16 tiles per core
FP32 = mybir.dt.float32
U8 = mybir.dt.uint8


def build_bass(repeat: int = 1):
    nc = bacc.Bacc("TRN2", target_bir_lowering=False, debug=False,
                   num_devices=N_CORES)
    x_in = nc.dram_tensor("x", [ROWS, T], FP32, kind="ExternalInput").ap()
    pm_in = nc.dram_tensor("pmask", [P, T], U8, kind="ExternalInput").ap()
    nm_in = nc.dram_tensor("nmask", [P, T], U8, kind="ExternalInput").ap()
    out = nc.dram_tensor("out", [ROWS, T], FP32, kind="ExternalOutput").ap()

    with tile.TileContext(nc) as tc:
        with tc.tile_pool(name="masks", bufs=1) as mpool, \
             tc.tile_pool(name="xin", bufs=3) as xpool, \
             tc.tile_pool(name="xout", bufs=3) as opool:
            pm = mpool.tile([P, T], U8, tag="pm")
            nc.sync.dma_start(pm[:], pm_in[:])
            nm = mpool.tile([P, T], U8, tag="nm")
            nc.sync.dma_start(nm[:], nm_in[:])
            for _ in range(repeat):
                for i in range(N_TILES):
                    xt = xpool.tile([P, T], FP32)
                    nc.sync.dma_start(xt[:], x_in[bass.ts(i, P), :])
                    ot = opool.tile([P, T], FP32)
                    nc.scalar.copy(ot[:], xt[:])
                    # left-neighbor replacements (t >= 1 only; g[0] != -1)
                    nc.vector.copy_predicated(
                        ot[:, bass.ds(1, T - 1)],
                        pm[:, bass.ds(1, T - 1)],
                        xt[:, bass.ds(0, T - 1)],
                    )
                    # right-neighbor replacements (t <= T-2 only)
                    nc.vector.copy_predicated(
                        ot[:, bass.ds(0, T - 1)],
                        nm[:, bass.ds(0, T - 1)],
                        xt[:, bass.ds(1, T - 1)],
                    )
                    nc.sync.dma_start(out[bass.ts(i, P), :], ot[:])
    nc.compile()
    return nc


def _host_masks(replace_mask: np.ndarray, neighbor_bits: np.ndarray):
    idx = np.arange(T)
    off = np.where(neighbor_bits > 0, 1, -1)
    nb = np.where(idx == 0, 1, np.where(idx == T - 1, T - 2, idx + off))
    g = np.where(replace_mask, nb, idx)
    pmask = (g == idx - 1).astype(np.uint8)
    nmask = (g == idx + 1).astype(np.uint8)
    pm_b = np.ascontiguousarray(np.broadcast_to(pmask, (P, T)))
    nm_b = np.ascontiguousarray(np.broadcast_to(nmask, (P, T)))
    return pm_b, nm_b


def kernel(x: np.ndarray, replace_mask: np.ndarray,
           neighbor_bits: np.ndarray) -> np.ndarray:
    x = np.asarray(x, dtype=np.float32)
    pm_b, nm_b = _host_masks(np.asarray(replace_mask),
                             np.asarray(neighbor_bits))
    nc = build_bass()
    in_maps = []
    for c in range(N_CORES):
        shard = np.ascontiguousarray(
            x[c * B_PER:(c + 1) * B_PER].reshape(ROWS, T))
        in_maps.append({"x": shard, "pmask": pm_b, "nmask": nm_b})
    res = run_bass_kernel_spmd(nc, in_maps, list(range(N_CORES))).results
    out = np.concatenate(
        [r["out"].reshape(B_PER, C, T) for r in res], axis=0)
    return np.ascontiguousarray(out)

